# revision 20
# baseline (speedup 1.0000x reference)
"""Trainium2 Bass kernel for the AutoregressiveSplineDeep flow.

Computes 4 steps of a MADE-conditioned monotonic linear-rational-spline flow
over N=131072 2-d samples, data-parallel over 8 NeuronCores.

Structure exploited: with input_dim=2 the MADE masks make the dim-0 spline
parameters constants (b3 even rows) and the dim-1 parameters a function of
the SCALAR z0 only; moreover z0 evolves by a FIXED scalar map F (the
constant dim-0 spline), so the dim-1 tables at step s are fixed functions
T(F^s(x0)) of the INITIAL x0.  All four steps' final per-bin stage-2
quantity tables (xk, yk, wc*yc, wb*yb, wk, lam*wk, wc, wb in diff form) are
tabulated on a 126-point x0 grid and encoded in an fp16 relu basis

  val(x0) = base + s0*u + sum_g (s_g - s_{g-1}) * relu(u - g),
  u = 12.5 x0 + 62.5

On device: ONE basis build (K=1 PE broadcast matmul of the x0 row + scalar
relu-with-bias -> rlu [128, ns] fp16), then per step per 128-sample chunk
one [128x128] PE matmul interpolates all 128 table columns into fp32 psum.
The scalar engine drains psum to SBUF, gpsimd does the per-sample masked
multiplies (ge' x diff-tables), DVE does the 16-wide dot reduces, the bin
compare (vs scanned cumulative widths) and the rational-spline arithmetic
in the wk-cancelled form  y = (yk*A + wcyc*dx) / (A + wc*dx)  (left branch,
mirrored right branch) -- no softplus/sigmoid/sqrt at runtime.
"""

import sys

sys.path.insert(0, "/opt/trn_rl_repo")

import numpy as np
import ml_dtypes

INPUT_DIM = 2
COUNT_BINS = 16
BOUND = 5.0
FLOW_LENGTH = 4
MIN_BIN = 1e-3
MIN_DERIV = 1e-3
MIN_LAMBDA = 0.025
N_FULL = 131072
N_CORES = 8
NS = N_FULL // N_CORES  # 16384 per core

LEFT, RIGHT = -BOUND, BOUND
CFREE = 1.0 - MIN_BIN * COUNT_BINS

GRID = 126              # x0 grid points, h = 0.08, u = 12.5*x0 + 62.5
NQ = 7                  # yk, wcyc, wbyb, wk, lamwk, wc, wb
NCOL = 128              # xkT(16) + 7q x 16


def _final_tables(raw):
    """raw [..., 63] -> (cw [..., 17], per-bin quantity tables) float64."""
    raw = raw.astype(np.float64)
    w = raw[..., 0:16]
    hh = raw[..., 16:32]
    dr = raw[..., 32:47]
    l = raw[..., 47:63]

    def smax(v):
        e = np.exp(v - v.max(-1, keepdims=True))
        return e / e.sum(-1, keepdims=True)

    widths = MIN_BIN + CFREE * smax(w)
    cw = np.concatenate([np.zeros_like(widths[..., :1]),
                         np.cumsum(widths, -1)], -1)
    cw = 2 * BOUND * cw - BOUND
    cw[..., 0] = -BOUND
    cw[..., -1] = BOUND
    wk = cw[..., 1:] - cw[..., :-1]
    heights = MIN_BIN + CFREE * smax(hh)
    ch = np.concatenate([np.zeros_like(heights[..., :1]),
                         np.cumsum(heights, -1)], -1)
    ch = 2 * BOUND * ch - BOUND
    ch[..., 0] = -BOUND
    ch[..., -1] = BOUND
    hk = ch[..., 1:] - ch[..., :-1]
    deriv = MIN_DERIV + np.log1p(np.exp(dr))
    one = np.ones_like(deriv[..., :1])
    deriv = np.concatenate([one, deriv, one], -1)
    lam = MIN_LAMBDA + (1 - 2 * MIN_LAMBDA) / (1 + np.exp(-l))
    dk = deriv[..., 0:16]
    dk1 = deriv[..., 1:17]
    wb = np.sqrt(dk / dk1)
    wc = (lam * dk + (1 - lam) * wb * dk1) * wk / hk
    yk = ch[..., 0:16]
    yb = yk + hk
    yc = ((1 - lam) * yk + lam * wb * yb) / ((1 - lam) + lam * wb)
    qs = dict(wk=wk, yk=yk, lamwk=lam * wk, wcyc=wc * yc, wbyb=wb * yb,
              wc=wc, wb=wb)
    return cw, qs


_QORDER = ["yk", "wcyc", "wbyb", "wk", "lamwk", "wc", "wb"]


def _dform(tab):
    return np.concatenate([tab[..., :1], tab[..., 1:] - tab[..., :-1]], -1)


def _spline0_exact(z, cw0, qs0):
    """Exact float64 dim-0 spline evaluation (for composing F^s on grid)."""
    xc = np.clip(z, -BOUND, BOUND)
    ge = (xc[:, None] >= cw0[0, 1:16][None, :]).astype(np.float64)
    gep = np.concatenate([np.ones((len(z), 1)), ge], 1)
    wf0 = cw0[0, 1:16] - cw0[0, 0:15]
    xk = (ge * wf0[None, :]).sum(1) - BOUND
    q = {k: (gep * _dform(qs0[k])[0][None, :]).sum(1) for k in qs0}
    dx = xc - xk
    A = q["lamwk"] - dx
    wkm = q["wk"] - dx
    lft = dx <= q["lamwk"]
    num = np.where(lft, q["yk"] * A + q["wcyc"] * dx,
                   q["wcyc"] * wkm - q["wbyb"] * A)
    den = np.where(lft, A + q["wc"] * dx, q["wc"] * wkm - q["wb"] * A)
    return np.where((z >= -BOUND) & (z <= BOUND), num / den, z)


def _np_tables(W1, b1, W2, b2, W3, b3):
    """Build CRELU [4, 128, NCOL] fp16 step tables + dim-0 const tables."""
    cw0, qs0 = _final_tables(b3[0::2][None, :])

    def mlp_cols(z):
        h1 = np.maximum(z[:, None] * W1[None, :, 0].astype(np.float64)
                        + b1[None, :].astype(np.float64), 0.0)
        h2 = np.maximum(h1 @ W2.T.astype(np.float64)
                        + b2.astype(np.float64), 0.0)
        raw1 = (h2 @ W3[1::2].T.astype(np.float64)
                + b3[1::2].astype(np.float64))
        cwg, qsg = _final_tables(raw1)
        cols = np.empty((len(z), NCOL))
        cols[:, 0] = 0.0
        cols[:, 1:16] = cwg[:, 1:16] - cwg[:, 0:15]     # wf[0..14]
        for i, q in enumerate(_QORDER):
            cols[:, 16 + 16 * i:16 + 16 * (i + 1)] = _dform(qsg[q])
        return cols

    def encode(cols):
        f16 = np.float16
        slopes = cols[1:] - cols[:-1]                   # [GRID-1, NCOL]
        base = cols[0]
        bhi = base.astype(f16).astype(np.float64)
        blo = (base - bhi).astype(f16).astype(np.float64)
        s0hi = slopes[0].astype(f16).astype(np.float64)
        s0lo = (slopes[0] - s0hi).astype(f16).astype(np.float64)
        M = np.zeros((128, NCOL), np.float32)
        M[0], M[1], M[2], M[3] = bhi, blo, s0hi, s0lo
        M[4:4 + GRID - 2] = slopes[1:] - slopes[:-1]    # rows 4..127
        return M

    zg = -BOUND + 0.08 * np.arange(GRID)
    crelu = np.empty((FLOW_LENGTH, 128, NCOL), np.float16)
    zz = zg.copy()
    for s in range(FLOW_LENGTH):
        crelu[s] = encode(mlp_cols(zz)).astype(np.float16)
        zz = _spline0_exact(zz, cw0, qs0)

    # dim-0 const tables, same 8x16 layout
    t0c8 = np.empty((8, 16), np.float64)
    t0c8[0, 0] = 0.0
    t0c8[0, 1:16] = cw0[0, 1:16] - cw0[0, 0:15]
    for i, q in enumerate(_QORDER):
        t0c8[1 + i] = _dform(qs0[q])[0]
    t0cmp = np.empty(16, np.float64)
    t0cmp[0] = -BOUND
    t0cmp[1:16] = cw0[0, 1:16]
    return crelu, t0c8.astype(np.float32), t0cmp.astype(np.float32)


def _build_program(ns):
    import concourse.bacc as bacc
    import concourse.tile as tile
    import concourse.mybir as mybir

    F32 = mybir.dt.float32
    BF16 = mybir.dt.bfloat16
    F16 = mybir.dt.float16

    CC = ns // 128
    G = 8
    NB = CC // G

    nc = bacc.Bacc("TRN2", target_bir_lowering=False, debug=False,
                   num_devices=N_CORES)

    def din(name, shape, dt=F32):
        return nc.dram_tensor(name, list(shape), dt, kind="ExternalInput").ap()

    def dout(name, shape, dt=F32):
        return nc.dram_tensor(name, list(shape), dt,
                              kind="ExternalOutput").ap()

    t = dict(
        ns=ns, CC=CC, G=G, NB=NB,
        xsp=din("XSP", [128, CC, 2]),
        x0rowb=din("X0ROWB", [1, ns], BF16),
        crelu=din("CRELU", [128, FLOW_LENGTH, NCOL], F16),
        arow=din("AROW", [1, 128], BF16),
        bvec=din("BVEC", [128, 1]),
        t0c8=din("T0C8", [128, 8, 16]),
        t0cmp=din("T0CMP", [128, 16]),
        scanm=din("SCANM", [128, G * 16]),
        z0out=dout("Z0OUT", [FLOW_LENGTH, 128, CC]),
        z1out=dout("Z1OUT", [FLOW_LENGTH, 128, CC]),
    )

    with tile.TileContext(nc) as tc:
        _emit(nc, tc, t)
    nc.compile()
    return nc


def _emit(nc, tc, t):
    import concourse.mybir as mybir
    import contextlib

    F32 = mybir.dt.float32
    F16 = mybir.dt.float16
    U8 = mybir.dt.uint8
    AF = mybir.ActivationFunctionType
    ALU = mybir.AluOpType
    AX = mybir.AxisListType.X

    ns, CC, G, NB = t["ns"], t["CC"], t["G"], t["NB"]

    ctx = contextlib.ExitStack()
    with ctx:
        consts = ctx.enter_context(tc.tile_pool(name="consts", bufs=1))
        zpool = ctx.enter_context(tc.tile_pool(name="z", bufs=1))
        basisp = ctx.enter_context(tc.tile_pool(name="basis", bufs=1))
        gep = ctx.enter_context(tc.tile_pool(name="ge", bufs=2))
        scr = ctx.enter_context(tc.tile_pool(name="scr", bufs=2))
        tbp = ctx.enter_context(tc.tile_pool(name="tb", bufs=2))
        s2p = ctx.enter_context(tc.tile_pool(name="s2", bufs=2))
        bcps = ctx.enter_context(tc.tile_pool(name="bc", bufs=2,
                                              space="PSUM"))
        gaps = ctx.enter_context(tc.tile_pool(name="ga", bufs=2,
                                              space="PSUM"))

        def cload(ap, shape, dt=F32):
            tl = consts.tile(shape, dt, tag=ap.name, name=ap.name)
            nc.sync.dma_start(tl[:], ap[:])
            return tl

        cCRELU = cload(t["crelu"], [128, FLOW_LENGTH, NCOL], F16)
        cAROW = cload(t["arow"], [1, 128], mybir.dt.bfloat16)
        cBVEC = cload(t["bvec"], [128, 1])
        cT0C8 = cload(t["t0c8"], [128, 8, 16])
        cT0CMP = cload(t["t0cmp"], [128, 16])
        cSCANM = cload(t["scanm"], [128, G * 16])

        z0t = [zpool.tile([128, CC], F32, tag=f"z0_{s}", name=f"z0_{s}")
               for s in range(FLOW_LENGTH + 1)]
        nc.sync.dma_start(z0t[0][:], t["xsp"][:, :, 0])
        z1t = [zpool.tile([128, CC], F32, tag=f"z1_{s}", name=f"z1_{s}")
               for s in range(FLOW_LENGTH + 1)]
        nc.sync.dma_start(z1t[0][:], t["xsp"][:, :, 1])
        x0row = basisp.tile([1, ns], mybir.dt.bfloat16, tag="x0row",
                            name="x0row")
        nc.sync.dma_start(x0row[:], t["x0rowb"][:])

        # ---- one-time relu basis over x0 ------------------------------
        rlu = basisp.tile([128, ns], F16, tag="rlu", name="rlu")
        for b in range(NB):
            bc = bcps.tile([128, G * 128], F32, tag="bc", name="bc")
            for hh in (0, 1):
                lo = G * 128 * b + 512 * hh
                nc.tensor.matmul(bc[:, 512 * hh:512 * (hh + 1)],
                                 cAROW[:], x0row[0:1, lo:lo + 512],
                                 start=True, stop=True)
            nc.scalar.activation(rlu[:, G * 128 * b:G * 128 * (b + 1)],
                                 bc[:], AF.Relu, bias=cBVEC[:])

        # ---- stage 2: rational spline from gathered coeffs ------------
        def stage2(qv, xcp5, xc, zin, zout, tagp):
            def tl(tag, dt=F32):
                return s2p.tile([128, CC], dt, tag=tagp + tag,
                                name=tagp + tag)

            # qv: xkd(0) yk(1) wcyc(2) wbyb(3) wk(4) lamwk(5) wc(6) wb(7)
            q = [qv[:, :, i] for i in range(8)]
            dx = tl("dx")
            nc.vector.tensor_tensor(dx[:], xcp5, q[0], ALU.subtract)
            av = tl("A")
            nc.vector.tensor_tensor(av[:], q[5], dx[:], ALU.subtract)
            wkmdx = tl("wkmdx")
            nc.gpsimd.tensor_tensor(wkmdx[:], q[4], dx[:], ALU.subtract)
            mleft = tl("ml", U8)
            nc.vector.tensor_tensor(mleft[:], dx[:], q[5], ALU.is_le)

            t1 = tl("t1")
            nc.gpsimd.tensor_tensor(t1[:], q[1], av[:], ALU.mult)
            t2 = tl("t2")
            nc.gpsimd.tensor_tensor(t2[:], q[2], dx[:], ALU.mult)
            numl = tl("numl")
            nc.vector.tensor_tensor(numl[:], t1[:], t2[:], ALU.add)
            t3 = tl("t3")
            nc.gpsimd.tensor_tensor(t3[:], q[6], dx[:], ALU.mult)
            denl = tl("denl")
            nc.vector.tensor_tensor(denl[:], av[:], t3[:], ALU.add)
            t4 = tl("t4")
            nc.gpsimd.tensor_tensor(t4[:], q[2], wkmdx[:], ALU.mult)
            t5 = tl("t5")
            nc.gpsimd.tensor_tensor(t5[:], q[3], av[:], ALU.mult)
            numr = tl("numr")
            nc.vector.tensor_tensor(numr[:], t4[:], t5[:], ALU.subtract)
            t6 = tl("t6")
            nc.gpsimd.tensor_tensor(t6[:], q[6], wkmdx[:], ALU.mult)
            t7 = tl("t7")
            nc.gpsimd.tensor_tensor(t7[:], q[7], av[:], ALU.mult)
            denr = tl("denr")
            nc.vector.tensor_tensor(denr[:], t6[:], t7[:], ALU.subtract)

            num = tl("num")
            nc.vector.tensor_copy(num[:], numr[:])
            nc.vector.copy_predicated(num[:], mleft[:], numl[:])
            den = tl("den")
            nc.gpsimd.tensor_copy(den[:], denr[:])
            nc.vector.copy_predicated(den[:], mleft[:], denl[:])
            rden = tl("rden")
            nc.vector.reciprocal_approx_fast(rden[:], den[:])
            y = tl("y")
            nc.gpsimd.tensor_tensor(y[:], num[:], rden[:], ALU.mult)
            mins = tl("mi", U8)
            nc.vector.tensor_tensor(mins[:], xc, zin, ALU.is_equal)
            nc.gpsimd.tensor_copy(zout, zin)
            nc.vector.copy_predicated(zout, mins[:], y[:])

        # ---- the four flow steps --------------------------------------
        for s in range(FLOW_LENGTH):
            # ======== dim 0 (constant tables) ========
            xc0 = s2p.tile([128, CC], F32, tag="a_xc", name="a_xc")
            nc.vector.tensor_scalar(xc0[:], z0t[s][:], LEFT, RIGHT,
                                    ALU.max, ALU.min)
            xcp50 = s2p.tile([128, CC], F32, tag="a_xcp5", name="a_xcp5")
            nc.vector.tensor_scalar(xcp50[:], xc0[:], BOUND, None, ALU.add)
            ge0h = gep.tile([128, CC, 16], F16, tag="ge0h", name="ge0h")
            nc.vector.tensor_tensor(
                ge0h[:],
                xc0[:].unsqueeze(2).broadcast_to((128, CC, 16)),
                cT0CMP[:].unsqueeze(1).broadcast_to((128, CC, 16)),
                ALU.is_ge)
            qv0 = s2p.tile([128, CC, 8], F32, tag="a_qv", name="a_qv")
            MB = 32
            for mb in range(CC // MB):
                sl = slice(MB * mb, MB * (mb + 1))
                s0m = scr.tile([128, MB, 8, 16], F32, tag="a_m", name="a_m")
                nc.gpsimd.tensor_tensor(
                    s0m[:],
                    cT0C8[:].unsqueeze(1).broadcast_to((128, MB, 8, 16)),
                    ge0h[:, sl, :].unsqueeze(2).broadcast_to(
                        (128, MB, 8, 16)),
                    ALU.mult)
                nc.vector.tensor_reduce(qv0[:, sl, :], s0m[:], AX, ALU.add)
            stage2(qv0[:], xcp50[:], xc0[:], z0t[s][:], z0t[s + 1][:], "a")
            nc.sync.dma_start(t["z0out"][s], z0t[s + 1][:])

            # ======== dim 1 (grid tables of x0, step matrix s) ========
            xc1 = s2p.tile([128, CC], F32, tag="b_xc", name="b_xc")
            nc.vector.tensor_scalar(xc1[:], z1t[s][:], LEFT, RIGHT,
                                    ALU.max, ALU.min)
            xcp51 = s2p.tile([128, CC], F32, tag="b_xcp5", name="b_xcp5")
            nc.vector.tensor_scalar(xcp51[:], xc1[:], BOUND, None, ALU.add)
            ge1h = gep.tile([128, CC, 16], F16, tag="ge1h", name="ge1h")
            qv1 = s2p.tile([128, CC, 8], F32, tag="b_qv", name="b_qv")

            for b in range(NB):
                sl = slice(G * b, G * b + G)
                gp = gaps.tile([128, G, 128], F32, tag="gp", name="gp")
                for g in range(G):
                    nc.tensor.matmul(
                        gp[:, g, :],
                        rlu[:, 128 * (G * b + g):128 * (G * b + g + 1)],
                        cCRELU[:, s, :], start=True, stop=True)
                tb = tbp.tile([128, G, 128], F32, tag="tb", name="tb")
                nc.scalar.copy(tb[:], gp[:])
                wfc = scr.tile([128, G * 16], F32, tag="b_wfc",
                               name="b_wfc")
                nc.gpsimd.tensor_copy(
                    wfc[:].rearrange("p (g c) -> p g c", c=16),
                    tb[:, :, 0:16])
                cwp = scr.tile([128, G * 16], F32, tag="b_cwp",
                               name="b_cwp")
                nc.vector.tensor_tensor_scan(
                    cwp[:], cSCANM[:], wfc[:], 0.0, ALU.mult, ALU.add)
                nc.vector.tensor_tensor(
                    ge1h[:, sl, :],
                    xcp51[:, sl].unsqueeze(2).broadcast_to((128, G, 16)),
                    cwp[:].rearrange("p (g c) -> p g c", c=16), ALU.is_ge)
                s1m = scr.tile([128, G, 8, 16], F32, tag="b_m", name="b_m")
                nc.gpsimd.tensor_tensor(
                    s1m[:],
                    tb[:].rearrange("p g (q k) -> p g q k", k=16),
                    ge1h[:, sl, :].unsqueeze(2).broadcast_to(
                        (128, G, 8, 16)),
                    ALU.mult)
                nc.vector.tensor_reduce(qv1[:, sl, :], s1m[:], AX, ALU.add)

            stage2(qv1[:], xcp51[:], xc1[:], z1t[s][:], z1t[s + 1][:], "b")
            nc.sync.dma_start(t["z1out"][s], z1t[s + 1][:])


_NC_CACHE = {}


def _get_program(ns):
    if ns not in _NC_CACHE:
        _NC_CACHE[ns] = _build_program(ns)
    return _NC_CACHE[ns]


def _make_inputs(x, W1, b1, W2, b2, W3, b3, ns):
    bf = ml_dtypes.bfloat16
    CC = ns // 128
    G = 8
    n_cores = x.shape[0] // ns

    crelu_steps, t0c8_v, t0cmp_v = _np_tables(W1, b1, W2, b2, W3, b3)
    # device layout [128, 4, NCOL]
    crelu = np.ascontiguousarray(
        crelu_steps.transpose(1, 0, 2)).astype(np.float16)

    # basis rows: 0,1 const(=relu(0+1)); 2,3 = u; 4+g-1 = relu(u-g), g=1..124
    arow = np.zeros((1, 128), np.float32)
    arow[0, 2:128] = 12.5
    bvec = np.zeros((128, 1), np.float32)
    bvec[0, 0] = 1.0
    bvec[1, 0] = 1.0
    bvec[2, 0] = 62.5
    bvec[3, 0] = 62.5
    g_idx = np.arange(1, GRID - 1, dtype=np.float32)
    bvec[4:4 + GRID - 2, 0] = 62.5 - g_idx

    scanm = np.tile(np.r_[0.0, np.ones(15)].astype(np.float32), G)

    shared = dict(
        CRELU=crelu,
        AROW=arow.astype(bf),
        BVEC=bvec,
        T0C8=np.broadcast_to(t0c8_v, (128, 8, 16)).copy(),
        T0CMP=np.broadcast_to(t0cmp_v, (128, 16)).copy(),
        SCANM=np.broadcast_to(scanm, (128, G * 16)).copy(),
    )

    in_maps = []
    for c in range(n_cores):
        xs = x[c * ns:(c + 1) * ns]
        xspc = xs.reshape(CC, 128, 2).transpose(1, 0, 2).copy()
        x0rowb = xs[:, 0].astype(bf)[None, :].copy()
        in_maps.append(dict(XSP=xspc.astype(np.float32), X0ROWB=x0rowb,
                            **shared))
    return in_maps


def _run(x, W1, b1, W2, b2, W3, b3, ns, trace=False):
    from concourse.bass_utils import run_bass_kernel_spmd

    n_cores = x.shape[0] // ns
    nc = _get_program(ns)
    in_maps = _make_inputs(x, W1, b1, W2, b2, W3, b3, ns)
    res = run_bass_kernel_spmd(nc, in_maps, list(range(n_cores)), trace=trace)

    n = x.shape[0]
    zs = np.empty((FLOW_LENGTH + 1, n, 2), np.float32)
    zs[0] = x
    for c in range(n_cores):
        r = res.results[c]
        lo = c * ns
        for s in range(FLOW_LENGTH):
            zs[s + 1, lo:lo + ns, 0] = r["Z0OUT"][s].T.reshape(ns)
            zs[s + 1, lo:lo + ns, 1] = r["Z1OUT"][s].T.reshape(ns)
    return zs, res


def kernel(x, W1, b1, W2, b2, W3, b3):
    x = np.ascontiguousarray(np.asarray(x, dtype=np.float32))
    zs, _ = _run(x, np.asarray(W1, np.float32), np.asarray(b1, np.float32),
                 np.asarray(W2, np.float32), np.asarray(b2, np.float32),
                 np.asarray(W3, np.float32), np.asarray(b3, np.float32),
                 NS)
    return zs


# revision 23
# speedup vs baseline: 1.0281x; 1.0281x over previous
"""Trainium2 Bass kernel for the AutoregressiveSplineDeep flow.

Computes 4 steps of a MADE-conditioned monotonic linear-rational-spline flow
over N=131072 2-d samples, data-parallel over 8 NeuronCores.

Structure exploited: with input_dim=2 the MADE masks make the dim-0 spline
parameters constants (b3 even rows) and the dim-1 parameters a function of
the SCALAR z0 only; moreover z0 evolves by a FIXED scalar map F (the
constant dim-0 spline), so the dim-1 tables at step s are fixed functions
T(F^s(x0)) of the INITIAL x0.  All four steps' final per-bin stage-2
quantity tables (xk, yk, wc*yc, wb*yb, wk, lam*wk, wc, wb in diff form) are
tabulated on a 126-point x0 grid and encoded in an fp16 relu basis

  val(x0) = base + s0*u + sum_g (s_g - s_{g-1}) * relu(u - g),
  u = 12.5 x0 + 62.5

On device: ONE basis build (K=1 PE broadcast matmul of the x0 row + scalar
relu-with-bias -> rlu [128, ns] fp16), then per step per 128-sample chunk
one [128x128] PE matmul interpolates all 128 table columns into fp32 psum.
The scalar engine drains psum to SBUF, gpsimd does the per-sample masked
multiplies (ge' x diff-tables), DVE does the 16-wide dot reduces, the bin
compare (vs scanned cumulative widths) and the rational-spline arithmetic
in the wk-cancelled form  y = (yk*A + wcyc*dx) / (A + wc*dx)  (left branch,
mirrored right branch) -- no softplus/sigmoid/sqrt at runtime.
"""

import sys

sys.path.insert(0, "/opt/trn_rl_repo")

import numpy as np
import ml_dtypes

INPUT_DIM = 2
COUNT_BINS = 16
BOUND = 5.0
FLOW_LENGTH = 4
MIN_BIN = 1e-3
MIN_DERIV = 1e-3
MIN_LAMBDA = 0.025
N_FULL = 131072
N_CORES = 8
NS = N_FULL // N_CORES  # 16384 per core

LEFT, RIGHT = -BOUND, BOUND
CFREE = 1.0 - MIN_BIN * COUNT_BINS

GRID = 126              # x0 grid points, h = 0.08, u = 12.5*x0 + 62.5
NQ = 7                  # yk, wcyc, wbyb, wk, lamwk, wc, wb
NCOL = 128              # xkT(16) + 7q x 16


def _final_tables(raw):
    """raw [..., 63] -> (cw [..., 17], per-bin quantity tables) float64."""
    raw = raw.astype(np.float64)
    w = raw[..., 0:16]
    hh = raw[..., 16:32]
    dr = raw[..., 32:47]
    l = raw[..., 47:63]

    def smax(v):
        e = np.exp(v - v.max(-1, keepdims=True))
        return e / e.sum(-1, keepdims=True)

    widths = MIN_BIN + CFREE * smax(w)
    cw = np.concatenate([np.zeros_like(widths[..., :1]),
                         np.cumsum(widths, -1)], -1)
    cw = 2 * BOUND * cw - BOUND
    cw[..., 0] = -BOUND
    cw[..., -1] = BOUND
    wk = cw[..., 1:] - cw[..., :-1]
    heights = MIN_BIN + CFREE * smax(hh)
    ch = np.concatenate([np.zeros_like(heights[..., :1]),
                         np.cumsum(heights, -1)], -1)
    ch = 2 * BOUND * ch - BOUND
    ch[..., 0] = -BOUND
    ch[..., -1] = BOUND
    hk = ch[..., 1:] - ch[..., :-1]
    deriv = MIN_DERIV + np.log1p(np.exp(dr))
    one = np.ones_like(deriv[..., :1])
    deriv = np.concatenate([one, deriv, one], -1)
    lam = MIN_LAMBDA + (1 - 2 * MIN_LAMBDA) / (1 + np.exp(-l))
    dk = deriv[..., 0:16]
    dk1 = deriv[..., 1:17]
    wb = np.sqrt(dk / dk1)
    wc = (lam * dk + (1 - lam) * wb * dk1) * wk / hk
    yk = ch[..., 0:16]
    yb = yk + hk
    yc = ((1 - lam) * yk + lam * wb * yb) / ((1 - lam) + lam * wb)
    qs = dict(wk=wk, yk=yk, lamwk=lam * wk, wcyc=wc * yc, wbyb=wb * yb,
              wc=wc, wb=wb)
    return cw, qs


_QORDER = ["yk", "wcyc", "wbyb", "wk", "lamwk", "wc", "wb"]


def _dform(tab):
    return np.concatenate([tab[..., :1], tab[..., 1:] - tab[..., :-1]], -1)


def _spline0_exact(z, cw0, qs0):
    """Exact float64 dim-0 spline evaluation (for composing F^s on grid)."""
    xc = np.clip(z, -BOUND, BOUND)
    ge = (xc[:, None] >= cw0[0, 1:16][None, :]).astype(np.float64)
    gep = np.concatenate([np.ones((len(z), 1)), ge], 1)
    wf0 = cw0[0, 1:16] - cw0[0, 0:15]
    xk = (ge * wf0[None, :]).sum(1) - BOUND
    q = {k: (gep * _dform(qs0[k])[0][None, :]).sum(1) for k in qs0}
    dx = xc - xk
    A = q["lamwk"] - dx
    wkm = q["wk"] - dx
    lft = dx <= q["lamwk"]
    num = np.where(lft, q["yk"] * A + q["wcyc"] * dx,
                   q["wcyc"] * wkm - q["wbyb"] * A)
    den = np.where(lft, A + q["wc"] * dx, q["wc"] * wkm - q["wb"] * A)
    return np.where((z >= -BOUND) & (z <= BOUND), num / den, z)


def _np_tables(W1, b1, W2, b2, W3, b3):
    """Build CRELU [4, 128, NCOL] fp16 step tables + dim-0 const tables."""
    cw0, qs0 = _final_tables(b3[0::2][None, :])

    def mlp_cols(z):
        h1 = np.maximum(z[:, None] * W1[None, :, 0].astype(np.float64)
                        + b1[None, :].astype(np.float64), 0.0)
        h2 = np.maximum(h1 @ W2.T.astype(np.float64)
                        + b2.astype(np.float64), 0.0)
        raw1 = (h2 @ W3[1::2].T.astype(np.float64)
                + b3[1::2].astype(np.float64))
        cwg, qsg = _final_tables(raw1)
        cols = np.empty((len(z), NCOL))
        cols[:, 0] = 0.0                                # cw[0]+5
        cols[:, 1:16] = cwg[:, 1:16] + BOUND            # cw[1..15]+5
        for i, q in enumerate(_QORDER):
            cols[:, 16 + 16 * i:16 + 16 * (i + 1)] = _dform(qsg[q])
        return cols

    def encode(cols):
        f16 = np.float16
        slopes = cols[1:] - cols[:-1]                   # [GRID-1, NCOL]
        base = cols[0]
        bhi = base.astype(f16).astype(np.float64)
        blo = (base - bhi).astype(f16).astype(np.float64)
        s0hi = slopes[0].astype(f16).astype(np.float64)
        s0lo = (slopes[0] - s0hi).astype(f16).astype(np.float64)
        M = np.zeros((128, NCOL), np.float32)
        M[0], M[1], M[2], M[3] = bhi, blo, s0hi, s0lo
        M[4:4 + GRID - 2] = slopes[1:] - slopes[:-1]    # rows 4..127
        return M

    zg = -BOUND + 0.08 * np.arange(GRID)
    crelu = np.empty((FLOW_LENGTH, 128, NCOL), np.float16)
    zz = zg.copy()
    for s in range(FLOW_LENGTH):
        crelu[s] = encode(mlp_cols(zz)).astype(np.float16)
        zz = _spline0_exact(zz, cw0, qs0)

    # dim-0 const tables, same 8x16 layout; group 0 = cw values (cmp + xk)
    t0c8 = np.empty((8, 16), np.float64)
    t0c8[0] = cw0[0, 0:16]
    for i, q in enumerate(_QORDER):
        t0c8[1 + i] = _dform(qs0[q])[0]
    return crelu, t0c8.astype(np.float32)


def _build_program(ns):
    import concourse.bacc as bacc
    import concourse.tile as tile
    import concourse.mybir as mybir

    F32 = mybir.dt.float32
    BF16 = mybir.dt.bfloat16
    F16 = mybir.dt.float16

    CC = ns // 128
    G = 8
    NB = CC // G

    nc = bacc.Bacc("TRN2", target_bir_lowering=False, debug=False,
                   num_devices=N_CORES)

    def din(name, shape, dt=F32):
        return nc.dram_tensor(name, list(shape), dt, kind="ExternalInput").ap()

    def dout(name, shape, dt=F32):
        return nc.dram_tensor(name, list(shape), dt,
                              kind="ExternalOutput").ap()

    t = dict(
        ns=ns, CC=CC, G=G, NB=NB,
        xsp=din("XSP", [128, CC, 2]),
        x0rowb=din("X0ROWB", [1, ns], BF16),
        crelu=din("CRELU", [128, FLOW_LENGTH, NCOL], F16),
        arow=din("AROW", [1, 128], BF16),
        bvec=din("BVEC", [128, 1]),
        t0c8=din("T0C8", [128, 8, 16]),
        z0out=dout("Z0OUT", [FLOW_LENGTH, 128, CC]),
        z1out=dout("Z1OUT", [FLOW_LENGTH, 128, CC]),
    )

    with tile.TileContext(nc) as tc:
        _emit(nc, tc, t)
    nc.compile()
    return nc


def _emit(nc, tc, t):
    import concourse.mybir as mybir
    import contextlib

    F32 = mybir.dt.float32
    F16 = mybir.dt.float16
    U8 = mybir.dt.uint8
    AF = mybir.ActivationFunctionType
    ALU = mybir.AluOpType
    AX = mybir.AxisListType.X

    ns, CC, G, NB = t["ns"], t["CC"], t["G"], t["NB"]

    ctx = contextlib.ExitStack()
    with ctx:
        consts = ctx.enter_context(tc.tile_pool(name="consts", bufs=1))
        zpool = ctx.enter_context(tc.tile_pool(name="z", bufs=1))
        basisp = ctx.enter_context(tc.tile_pool(name="basis", bufs=1))
        gep = ctx.enter_context(tc.tile_pool(name="ge", bufs=2))
        scr = ctx.enter_context(tc.tile_pool(name="scr", bufs=2))
        tbp = ctx.enter_context(tc.tile_pool(name="tb", bufs=2))
        s2p = ctx.enter_context(tc.tile_pool(name="s2", bufs=2))
        bcps = ctx.enter_context(tc.tile_pool(name="bc", bufs=2,
                                              space="PSUM"))
        gaps = ctx.enter_context(tc.tile_pool(name="ga", bufs=2,
                                              space="PSUM"))

        def cload(ap, shape, dt=F32):
            tl = consts.tile(shape, dt, tag=ap.name, name=ap.name)
            nc.sync.dma_start(tl[:], ap[:])
            return tl

        cCRELU = cload(t["crelu"], [128, FLOW_LENGTH, NCOL], F16)
        cAROW = cload(t["arow"], [1, 128], mybir.dt.bfloat16)
        cBVEC = cload(t["bvec"], [128, 1])
        cT0C8 = cload(t["t0c8"], [128, 8, 16])

        z0t = [zpool.tile([128, CC], F32, tag=f"z0_{s}", name=f"z0_{s}")
               for s in range(FLOW_LENGTH + 1)]
        nc.sync.dma_start(z0t[0][:], t["xsp"][:, :, 0])
        z1t = [zpool.tile([128, CC], F32, tag=f"z1_{s}", name=f"z1_{s}")
               for s in range(FLOW_LENGTH + 1)]
        nc.sync.dma_start(z1t[0][:], t["xsp"][:, :, 1])
        x0row = basisp.tile([1, ns], mybir.dt.bfloat16, tag="x0row",
                            name="x0row")
        nc.sync.dma_start(x0row[:], t["x0rowb"][:])

        # ---- one-time relu basis over x0 ------------------------------
        rlu = basisp.tile([128, ns], F16, tag="rlu", name="rlu")
        for b in range(NB):
            bc = bcps.tile([128, G * 128], F32, tag="bc", name="bc")
            for hh in (0, 1):
                lo = G * 128 * b + 512 * hh
                nc.tensor.matmul(bc[:, 512 * hh:512 * (hh + 1)],
                                 cAROW[:], x0row[0:1, lo:lo + 512],
                                 start=True, stop=True)
            nc.scalar.activation(rlu[:, G * 128 * b:G * 128 * (b + 1)],
                                 bc[:], AF.Relu, bias=cBVEC[:])

        # ---- stage 2: rational spline from gathered coeffs ------------
        def stage2(qv, xcp5, xc, zin, zout, tagp):
            def tl(tag, dt=F32):
                return s2p.tile([128, CC], dt, tag=tagp + tag,
                                name=tagp + tag)

            # qv: xkd(0) yk(1) wcyc(2) wbyb(3) wk(4) lamwk(5) wc(6) wb(7)
            q = [qv[:, :, i] for i in range(8)]
            dx = tl("dx")
            nc.vector.tensor_tensor(dx[:], xcp5, q[0], ALU.subtract)
            av = tl("A")
            nc.vector.tensor_tensor(av[:], q[5], dx[:], ALU.subtract)
            wkmdx = tl("wkmdx")
            nc.gpsimd.tensor_tensor(wkmdx[:], q[4], dx[:], ALU.subtract)
            mleft = tl("ml", U8)
            nc.vector.tensor_tensor(mleft[:], dx[:], q[5], ALU.is_le)

            t1 = tl("t1")
            nc.gpsimd.tensor_tensor(t1[:], q[1], av[:], ALU.mult)
            t2 = tl("t2")
            nc.gpsimd.tensor_tensor(t2[:], q[2], dx[:], ALU.mult)
            numl = tl("numl")
            nc.vector.tensor_tensor(numl[:], t1[:], t2[:], ALU.add)
            t3 = tl("t3")
            nc.gpsimd.tensor_tensor(t3[:], q[6], dx[:], ALU.mult)
            denl = tl("denl")
            nc.vector.tensor_tensor(denl[:], av[:], t3[:], ALU.add)
            t4 = tl("t4")
            nc.gpsimd.tensor_tensor(t4[:], q[2], wkmdx[:], ALU.mult)
            t5 = tl("t5")
            nc.gpsimd.tensor_tensor(t5[:], q[3], av[:], ALU.mult)
            numr = tl("numr")
            nc.vector.tensor_tensor(numr[:], t4[:], t5[:], ALU.subtract)
            t6 = tl("t6")
            nc.gpsimd.tensor_tensor(t6[:], q[6], wkmdx[:], ALU.mult)
            t7 = tl("t7")
            nc.gpsimd.tensor_tensor(t7[:], q[7], av[:], ALU.mult)
            denr = tl("denr")
            nc.vector.tensor_tensor(denr[:], t6[:], t7[:], ALU.subtract)

            num = tl("num")
            nc.vector.tensor_copy(num[:], numr[:])
            nc.vector.copy_predicated(num[:], mleft[:], numl[:])
            den = tl("den")
            nc.gpsimd.tensor_copy(den[:], denr[:])
            nc.vector.copy_predicated(den[:], mleft[:], denl[:])
            rden = tl("rden")
            nc.vector.reciprocal_approx_fast(rden[:], den[:])
            y = tl("y")
            nc.gpsimd.tensor_tensor(y[:], num[:], rden[:], ALU.mult)
            mins = tl("mi", U8)
            nc.vector.tensor_tensor(mins[:], xc, zin, ALU.is_equal)
            nc.gpsimd.tensor_copy(zout, zin)
            nc.vector.copy_predicated(zout, mins[:], y[:])

        # ---- the four flow steps --------------------------------------
        for s in range(FLOW_LENGTH):
            # ======== dim 0 (constant tables) ========
            xc0 = s2p.tile([128, CC], F32, tag="a_xc", name="a_xc")
            nc.vector.tensor_scalar(xc0[:], z0t[s][:], LEFT, RIGHT,
                                    ALU.max, ALU.min)
            ge0h = gep.tile([128, CC, 16], F16, tag="ge0h", name="ge0h")
            nc.vector.tensor_tensor(
                ge0h[:],
                xc0[:].unsqueeze(2).broadcast_to((128, CC, 16)),
                cT0C8[:, 0, :].unsqueeze(1).broadcast_to((128, CC, 16)),
                ALU.is_ge)
            oh0 = gep.tile([128, CC, 16], F16, tag="oh0", name="oh0")
            nc.gpsimd.tensor_tensor(oh0[:, :, 0:15], ge0h[:, :, 0:15],
                                    ge0h[:, :, 1:16], ALU.subtract)
            nc.gpsimd.tensor_copy(oh0[:, :, 15], ge0h[:, :, 15])
            qv0 = s2p.tile([128, CC, 8], F32, tag="a_qv", name="a_qv")
            MB = 32
            for mb in range(CC // MB):
                sl = slice(MB * mb, MB * (mb + 1))
                s0m = scr.tile([128, MB, 8, 16], F32, tag="a_m", name="a_m")
                eng = nc.gpsimd if mb % 2 == 0 else nc.vector
                eng.tensor_tensor(
                    s0m[:, :, 0, :],
                    cT0C8[:, 0, :].unsqueeze(1).broadcast_to((128, MB, 16)),
                    oh0[:, sl, :], ALU.mult)
                eng.tensor_tensor(
                    s0m[:, :, 1:8, :],
                    cT0C8[:, 1:8, :].unsqueeze(1).broadcast_to(
                        (128, MB, 7, 16)),
                    ge0h[:, sl, :].unsqueeze(2).broadcast_to(
                        (128, MB, 7, 16)),
                    ALU.mult)
                nc.vector.tensor_reduce(qv0[:, sl, :], s0m[:], AX, ALU.add)
            stage2(qv0[:], xc0[:], xc0[:], z0t[s][:], z0t[s + 1][:], "a")
            nc.sync.dma_start(t["z0out"][s], z0t[s + 1][:])

            # ======== dim 1 (grid tables of x0, step matrix s) ========
            xc1 = s2p.tile([128, CC], F32, tag="b_xc", name="b_xc")
            nc.vector.tensor_scalar(xc1[:], z1t[s][:], LEFT, RIGHT,
                                    ALU.max, ALU.min)
            xcp51 = s2p.tile([128, CC], F32, tag="b_xcp5", name="b_xcp5")
            nc.vector.tensor_scalar(xcp51[:], xc1[:], BOUND, None, ALU.add)
            ge1h = gep.tile([128, CC, 16], F16, tag="ge1h", name="ge1h")
            oh1 = gep.tile([128, CC, 16], F16, tag="oh1", name="oh1")
            qv1 = s2p.tile([128, CC, 8], F32, tag="b_qv", name="b_qv")

            for b in range(NB):
                sl = slice(G * b, G * b + G)
                gp = gaps.tile([128, G, 128], F32, tag="gp", name="gp")
                for g in range(G):
                    nc.tensor.matmul(
                        gp[:, g, :],
                        rlu[:, 128 * (G * b + g):128 * (G * b + g + 1)],
                        cCRELU[:, s, :], start=True, stop=True)
                tb = tbp.tile([128, G, 128], F32, tag="tb", name="tb")
                nc.scalar.copy(tb[:], gp[:])
                nc.vector.tensor_tensor(
                    ge1h[:, sl, :],
                    xcp51[:, sl].unsqueeze(2).broadcast_to((128, G, 16)),
                    gp[:, :, 0:16], ALU.is_ge)
                nc.gpsimd.tensor_tensor(oh1[:, sl, 0:15],
                                        ge1h[:, sl, 0:15],
                                        ge1h[:, sl, 1:16], ALU.subtract)
                nc.gpsimd.tensor_copy(oh1[:, sl, 15], ge1h[:, sl, 15])
                s1m = scr.tile([128, G, 8, 16], F32, tag="b_m", name="b_m")
                eng = nc.gpsimd if b % 2 == 0 else nc.vector
                eng.tensor_tensor(s1m[:, :, 0, :], tb[:, :, 0:16],
                                  oh1[:, sl, :], ALU.mult)
                eng.tensor_tensor(
                    s1m[:, :, 1:8, :],
                    tb[:, :, 16:128].rearrange("p g (q k) -> p g q k",
                                               k=16),
                    ge1h[:, sl, :].unsqueeze(2).broadcast_to(
                        (128, G, 7, 16)),
                    ALU.mult)
                nc.vector.tensor_reduce(qv1[:, sl, :], s1m[:], AX, ALU.add)

            stage2(qv1[:], xcp51[:], xc1[:], z1t[s][:], z1t[s + 1][:], "b")
            nc.sync.dma_start(t["z1out"][s], z1t[s + 1][:])


_NC_CACHE = {}


def _get_program(ns):
    if ns not in _NC_CACHE:
        _NC_CACHE[ns] = _build_program(ns)
    return _NC_CACHE[ns]


def _make_inputs(x, W1, b1, W2, b2, W3, b3, ns):
    bf = ml_dtypes.bfloat16
    CC = ns // 128
    G = 8
    n_cores = x.shape[0] // ns

    crelu_steps, t0c8_v = _np_tables(W1, b1, W2, b2, W3, b3)
    # device layout [128, 4, NCOL]
    crelu = np.ascontiguousarray(
        crelu_steps.transpose(1, 0, 2)).astype(np.float16)

    # basis rows: 0,1 const(=relu(0+1)); 2,3 = u; 4+g-1 = relu(u-g), g=1..124
    arow = np.zeros((1, 128), np.float32)
    arow[0, 2:128] = 12.5
    bvec = np.zeros((128, 1), np.float32)
    bvec[0, 0] = 1.0
    bvec[1, 0] = 1.0
    bvec[2, 0] = 62.5
    bvec[3, 0] = 62.5
    g_idx = np.arange(1, GRID - 1, dtype=np.float32)
    bvec[4:4 + GRID - 2, 0] = 62.5 - g_idx

    shared = dict(
        CRELU=crelu,
        AROW=arow.astype(bf),
        BVEC=bvec,
        T0C8=np.broadcast_to(t0c8_v, (128, 8, 16)).copy(),
    )

    in_maps = []
    for c in range(n_cores):
        xs = x[c * ns:(c + 1) * ns]
        xspc = xs.reshape(CC, 128, 2).transpose(1, 0, 2).copy()
        x0rowb = xs[:, 0].astype(bf)[None, :].copy()
        in_maps.append(dict(XSP=xspc.astype(np.float32), X0ROWB=x0rowb,
                            **shared))
    return in_maps


def _run(x, W1, b1, W2, b2, W3, b3, ns, trace=False):
    from concourse.bass_utils import run_bass_kernel_spmd

    n_cores = x.shape[0] // ns
    nc = _get_program(ns)
    in_maps = _make_inputs(x, W1, b1, W2, b2, W3, b3, ns)
    res = run_bass_kernel_spmd(nc, in_maps, list(range(n_cores)), trace=trace)

    n = x.shape[0]
    zs = np.empty((FLOW_LENGTH + 1, n, 2), np.float32)
    zs[0] = x
    for c in range(n_cores):
        r = res.results[c]
        lo = c * ns
        for s in range(FLOW_LENGTH):
            zs[s + 1, lo:lo + ns, 0] = r["Z0OUT"][s].T.reshape(ns)
            zs[s + 1, lo:lo + ns, 1] = r["Z1OUT"][s].T.reshape(ns)
    return zs, res


def kernel(x, W1, b1, W2, b2, W3, b3):
    x = np.ascontiguousarray(np.asarray(x, dtype=np.float32))
    zs, _ = _run(x, np.asarray(W1, np.float32), np.asarray(b1, np.float32),
                 np.asarray(W2, np.float32), np.asarray(b2, np.float32),
                 np.asarray(W3, np.float32), np.asarray(b3, np.float32),
                 NS)
    return zs
